# revision 13
# baseline (speedup 1.0000x reference)
"""ChebConv(K=3) x2 + Linear GNN on 8 Trainium2 NeuronCores.

Strategy (graph/data parallel, per sharding hint):
  - Nodes are sharded by range across 8 cores (6250 real nodes/core, padded
    to 6272 = 49*128).  Each core owns the segment-sum outputs for its node
    range.  Chebyshev weights are replicated.
  - prop(z) = -D^-1/2 A D^-1/2 z is factored as  -diag(dis) * M(diag(dis) z)
    where M is the unweighted gather-sum over edges.  Source features are
    pre-scaled by dis before each AllGather, so the per-edge weight reduces
    to -dis[col], which is folded into the one-hot scatter matrix.
  - Per prop, each core dma_gathers its edges' source rows (256B each) from
    the replicated full feature table in HBM, then scatter-adds them into
    PSUM with one-hot matmuls on the TensorEngine (one-hots built on the
    VectorEngine from iota == col_local, scaled by -dis[col]).
  - Between props, the updated (pre-scaled) node features are AllGathered
    (3 collectives total).
  - dma_gather indices are int16, so the source table is split in two halves
    (< 32768 rows each); each tile's edges are grouped by (tile, half).
  - The schedule is uniform across cores (SPMD): per (tile, half) groups are
    padded to a common chunk cap; pad slots carry weight 0.
"""

import math
import os

import numpy as np

os.environ.setdefault("MYCRO_LOCAL_CACHE", "1")

P = 128          # partitions / chunk size
D = 64           # feature dim
NCORE = 8

# Set by test harness to get an NTFF profile; leave False for grading.
TRACE = False
LAST_EXEC_NS = None
LAST_BUILD = None   # (nc, in_maps) of the most recent run, for benchmarking


# ----------------------------------------------------------------------------
# Host-side preprocessing
# ----------------------------------------------------------------------------

def _preprocess(x, edge_index):
    """Build per-core edge streams + device input arrays."""
    n = x.shape[0]
    npc = (n + NCORE - 1) // NCORE          # real nodes per core
    t_per_core = (npc + P - 1) // P         # tiles per core
    T = t_per_core
    S = T * P                               # padded nodes per core
    NPTOT = S * NCORE
    HALF = (NCORE // 2) * S
    assert HALF < 32768 and NPTOT - HALF < 32768

    row = np.asarray(edge_index[0], dtype=np.int64)
    col = np.asarray(edge_index[1], dtype=np.int64)
    E = row.shape[0]

    deg = np.bincount(row, minlength=n).astype(np.float64)
    dis = np.where(deg > 0, 1.0 / np.sqrt(np.maximum(deg, 1.0)), 0.0).astype(
        np.float32
    )

    # node -> (core, tile t, partition p); padded slice row r = p*T + t
    core_of = col // npc
    lc = col - core_of * npc
    t_of = lc % T
    p_of = lc // T
    # padded global id of the source node (row-major: core*S + local)
    g_src = (row // npc) * S + (row % npc)
    h_of = (g_src >= HALF).astype(np.int64)
    w_e = (-dis[row] * dis[col]).astype(np.float32)

    group = (core_of * T + t_of) * 2 + h_of          # global group id
    order = np.argsort(group, kind="stable")
    cnt = np.bincount(group, minlength=NCORE * T * 2)
    cap_chunks = max(1, int(math.ceil(cnt.max() / P)))
    CAPS = cap_chunks * P                             # slots per group
    nch = 2 * T * cap_chunks                          # chunks per core
    slots = nch * P                                   # slots per core

    gidx = np.zeros((NCORE, slots), np.int16)
    colv = np.zeros((NCORE, slots), np.float32)
    negv = np.zeros((NCORE, slots), np.float32)

    g_sorted = group[order]
    starts = np.zeros(NCORE * T * 2 + 1, np.int64)
    np.cumsum(cnt, out=starts[1:])
    pos = np.arange(E, dtype=np.int64) - starts[g_sorted]
    core_s = g_sorted // (2 * T)
    slot = (g_sorted % (2 * T)) * CAPS + pos
    gidx[core_s, slot] = (g_src - h_of * HALF)[order].astype(np.int16)
    colv[core_s, slot] = p_of[order].astype(np.float32)
    negv[core_s, slot] = w_e[order]

    # device layouts
    idx_dev = np.zeros((NCORE, 128, nch * 8), np.int16)
    for c in range(NCORE):
        blocks = gidx[c].reshape(2 * T, CAPS // 16, 16)   # [group, s, 16]
        wrapped = blocks.transpose(0, 2, 1).reshape(2 * T, 16, CAPS // 16)
        flat = np.concatenate(list(wrapped), axis=1)      # [16, nch*8]
        idx_dev[c] = np.tile(flat, (8, 1))
    colv_dev = colv.reshape(NCORE, nch, P).transpose(0, 2, 1).copy()
    negv_dev = negv.reshape(NCORE, nch, P).transpose(0, 2, 1).copy()

    # per-core dis table [128, T] and xT [64, T*128] (tile-major columns)
    dis_t = np.zeros((NCORE, P, T), np.float32)
    xT = np.zeros((NCORE, D, S), np.float32)
    xhat = np.zeros((NPTOT, D), np.float32)
    r_all = np.arange(npc, dtype=np.int64)
    t_r = r_all % T
    p_r = r_all // T
    for c in range(NCORE):
        nodes = c * npc + r_all
        valid = nodes < n
        nv = nodes[valid]
        dis_t[c, p_r[valid], t_r[valid]] = dis[nv]
        xT[c][:, t_r[valid] * P + p_r[valid]] = np.asarray(x)[nv].T
        xhat[c * S + r_all[valid]] = np.asarray(x)[nv]

    meta = dict(
        n=n, npc=npc, T=T, S=S, NPTOT=NPTOT, HALF=HALF,
        cap=cap_chunks, nch=nch,
    )
    return meta, dict(
        idx=idx_dev, colv=colv_dev, negv=negv_dev, dis_t=dis_t, xT=xT,
        xhat=xhat,
    )


# ----------------------------------------------------------------------------
# Device program
# ----------------------------------------------------------------------------

def _build_program(meta):
    import concourse.bacc as bacc
    import concourse.mybir as mybir
    from concourse.tile import TileContext

    T, S, NPTOT, HALF = meta["T"], meta["S"], meta["NPTOT"], meta["HALF"]
    CAP, NCH = meta["cap"], meta["nch"]
    f32 = mybir.dt.float32

    nc = bacc.Bacc("TRN2", target_bir_lowering=False, debug=False,
                   num_devices=NCORE)

    # I/O
    xhat_in = nc.dram_tensor("xhat", [NPTOT, D], f32, kind="ExternalInput")
    xT_in = nc.dram_tensor("xT", [D, S], f32, kind="ExternalInput")
    idx_in = nc.dram_tensor("idx", [128, NCH * 8], mybir.dt.int16,
                            kind="ExternalInput")
    colv_in = nc.dram_tensor("colv", [P, NCH], f32, kind="ExternalInput")
    negv_in = nc.dram_tensor("negv", [P, NCH], f32, kind="ExternalInput")
    dis_in = nc.dram_tensor("dis_t", [P, T], f32, kind="ExternalInput")
    iota_in = nc.dram_tensor("iota", [P, P], f32, kind="ExternalInput")
    ident_in = nc.dram_tensor("ident", [P, P], f32, kind="ExternalInput")
    w_in = {}
    for nm in ("w1a", "w1b", "w1c", "w2a", "w2b", "w2c"):
        w_in[nm] = nc.dram_tensor(nm, [D, D], f32, kind="ExternalInput")
    wlin_in = nc.dram_tensor("wlin", [D, 2], f32, kind="ExternalInput")
    b1_in = nc.dram_tensor("b1", [D, 1], f32, kind="ExternalInput")
    b2_in = nc.dram_tensor("b2", [D, 1], f32, kind="ExternalInput")
    blin_in = nc.dram_tensor("blin", [2, 1], f32, kind="ExternalInput")
    y_out = nc.dram_tensor("y", [2, S], f32, kind="ExternalOutput")

    # internal DRAM for collectives
    ag_in = [nc.dram_tensor(f"ag_in{i}", [S, D], f32) for i in range(3)]
    ag_out = [
        nc.dram_tensor(f"ag_out{i}", [NPTOT, D], f32, addr_space="Shared")
        for i in range(3)
    ]

    with TileContext(nc) as tc:
        const = tc.alloc_tile_pool(name="const", bufs=1)
        work = tc.alloc_tile_pool(name="work", bufs=2)
        oh_pool = tc.alloc_tile_pool(name="oh", bufs=4)
        psum = tc.alloc_tile_pool(name="psum", bufs=2, space="PSUM")
        psum1 = tc.alloc_tile_pool(name="psum1", bufs=1, space="PSUM")

        # persistent SBUF
        idx_sb = const.tile([128, NCH * 8], mybir.dt.int16)
        colv_sb = const.tile([P, NCH], f32)
        negv_sb = const.tile([P, NCH], f32)
        dis_sb = const.tile([P, T], f32)
        iota_sb = const.tile([P, P], f32)
        ident_sb = const.tile([P, P], f32)
        w_sb = {nm: const.tile([D, D], f32, name=f"w_{nm}") for nm in w_in}
        wlin_sb = const.tile([D, 2], f32)
        b1_sb = const.tile([D, 1], f32)
        b2_sb = const.tile([D, 1], f32)
        blin_sb = const.tile([2, 1], f32)
        tx1T_sb = const.tile([D, S], f32)
        h1T_sb = const.tile([D, S], f32)
        u_sb = const.tile([P, T * D], f32)
        logT_sb = const.tile([2, S], f32)

        nc.sync.dma_start(out=idx_sb[:], in_=idx_in[:])
        nc.sync.dma_start(out=colv_sb[:], in_=colv_in[:])
        nc.sync.dma_start(out=negv_sb[:], in_=negv_in[:])
        nc.sync.dma_start(out=dis_sb[:], in_=dis_in[:])
        nc.sync.dma_start(out=iota_sb[:], in_=iota_in[:])
        nc.sync.dma_start(out=ident_sb[:], in_=ident_in[:])
        for nm in w_in:
            nc.sync.dma_start(out=w_sb[nm][:], in_=w_in[nm][:])
        nc.sync.dma_start(out=wlin_sb[:], in_=wlin_in[:])
        nc.sync.dma_start(out=b1_sb[:], in_=b1_in[:])
        nc.sync.dma_start(out=b2_sb[:], in_=b2_in[:])
        nc.sync.dma_start(out=blin_sb[:], in_=blin_in[:])

        Copy = mybir.ActivationFunctionType.Copy
        Relu = mybir.ActivationFunctionType.Relu

        def scatter_prop_tile(src, t):
            """Gather + one-hot scatter matmul for tile t -> psum [128, 64].

            src: DRAM AP [NPTOT, D] to gather from (pre-scaled by dis).
            Returns the psum tile (= -dis[col] * sum(src[row])).
            """
            gbuf = work.tile([P, 2 * CAP, D], f32, tag="gbuf")
            lo = src[0:HALF, :]
            hi = src[HALF:NPTOT, :]
            base = (2 * t) * CAP * 8
            nc.gpsimd.dma_gather(
                out_ap=gbuf[:, 0:CAP, :], in_ap=lo,
                idxs_ap=idx_sb[:, base:base + CAP * 8],
                num_idxs=CAP * P, num_idxs_reg=CAP * P, elem_size=D,
                single_packet=False,
            )
            nc.gpsimd.dma_gather(
                out_ap=gbuf[:, CAP:2 * CAP, :], in_ap=hi,
                idxs_ap=idx_sb[:, base + CAP * 8:base + 2 * CAP * 8],
                num_idxs=CAP * P, num_idxs_reg=CAP * P, elem_size=D,
                single_packet=False,
            )
            ps = psum.tile([P, D], f32, tag="scatter")
            ch0 = 2 * t * CAP
            for k in range(2 * CAP):
                oh = oh_pool.tile([P, P], f32, tag="oh")
                nc.vector.tensor_scalar(
                    out=oh[:], in0=iota_sb[:],
                    scalar1=colv_sb[:, ch0 + k:ch0 + k + 1],
                    scalar2=negv_sb[:, ch0 + k:ch0 + k + 1],
                    op0=mybir.AluOpType.is_equal,
                    op1=mybir.AluOpType.mult,
                )
                nc.tensor.matmul(
                    out=ps[:], lhsT=oh[:], rhs=gbuf[:, k, :],
                    start=(k == 0), stop=(k == 2 * CAP - 1),
                )
            return ps

        def transpose_to(ps_nodemajor, t, dest_sb):
            """psum [128,64] (node-major) -> dest_sb[:, t*128:(t+1)*128]."""
            cp = work.tile([P, D], f32, tag="cp")
            nc.scalar.activation(out=cp[:], in_=ps_nodemajor[:], func=Copy)
            pst = psum.tile([D, P], f32, tag="trans")
            nc.tensor.transpose(out=pst[:], in_=cp[:], identity=ident_sb[:])
            nc.scalar.activation(
                out=dest_sb[:, t * P:(t + 1) * P], in_=pst[:], func=Copy)

        def layer_out_tile(t, wa, wb, wc, p2T_sb, xT_t, bias_sb, dest_sb):
            """dest_sb[:, tile t] = relu((wa.T@xT + wb.T@tx1T + wc.T@p2T) + b)."""
            po = psum.tile([D, P], f32, tag="lout")
            nc.tensor.matmul(out=po[:], lhsT=w_sb[wa][:], rhs=xT_t,
                             start=True, stop=False)
            nc.tensor.matmul(out=po[:], lhsT=w_sb[wb][:],
                             rhs=tx1T_sb[:, t * P:(t + 1) * P],
                             start=False, stop=False)
            nc.tensor.matmul(out=po[:], lhsT=w_sb[wc][:], rhs=p2T_sb[:],
                             start=False, stop=True)
            nc.scalar.activation(
                out=dest_sb[:, t * P:(t + 1) * P],
                in_=po[:], func=Relu, bias=bias_sb[:, 0:1])

        def prop_pass(src, to_txT, stage_u):
            """One prop over all tiles.

            to_txT: SBUF [64, S] to store transposed prop result, or None.
            stage_u: if True, stage dis*psum into u_sb (for AllGather).
            """
            for t in range(T):
                ps = scatter_prop_tile(src, t)
                if to_txT is not None:
                    transpose_to(ps, t, to_txT)
                if stage_u:
                    nc.scalar.activation(
                        out=u_sb[:, t * D:(t + 1) * D], in_=ps[:], func=Copy)

        def allgather(i):
            nc.sync.dma_start(
                out=ag_in[i][:].rearrange("(p t) f -> p (t f)", p=P),
                in_=u_sb[:])
            nc.gpsimd.collective_compute(
                "AllGather", mybir.AluOpType.bypass,
                replica_groups=[list(range(NCORE))],
                ins=[ag_in[i][:]], outs=[ag_out[i][:]],
            )

        # ---- layer 1 ----
        prop_pass(xhat_in[:], tx1T_sb, stage_u=True)
        allgather(0)
        # prop2 + layer-1 output, fused per tile
        for t in range(T):
            ps2 = scatter_prop_tile(ag_out[0][:], t)
            cp = work.tile([P, D], f32, tag="cp")
            nc.scalar.activation(out=cp[:], in_=ps2[:], func=Copy)
            pst = psum.tile([D, P], f32, tag="trans")
            nc.tensor.transpose(out=pst[:], in_=cp[:], identity=ident_sb[:])
            p2T = work.tile([D, P], f32, tag="p2T")
            nc.vector.tensor_copy(out=p2T[:], in_=pst[:])
            xT_t = work.tile([D, P], f32, tag="xTt")
            nc.sync.dma_start(out=xT_t[:], in_=xT_in[:, t * P:(t + 1) * P])
            layer_out_tile(t, "w1a", "w1b", "w1c", p2T, xT_t[:], b1_sb,
                           h1T_sb)
            # u1 = dis * h1 (node-major) for next AllGather
            psn = psum1.tile([P, D], f32, tag="nodemaj")
            nc.tensor.matmul(
                out=psn[:], lhsT=h1T_sb[:, t * P:(t + 1) * P],
                rhs=ident_sb[:D, :D], is_transpose=True)
            nc.scalar.activation(
                out=u_sb[:, t * D:(t + 1) * D], in_=psn[:], func=Copy)
        allgather(1)

        # ---- layer 2 ----
        prop_pass(ag_out[1][:], tx1T_sb, stage_u=True)
        allgather(2)
        for t in range(T):
            ps2 = scatter_prop_tile(ag_out[2][:], t)
            cp = work.tile([P, D], f32, tag="cp")
            nc.scalar.activation(out=cp[:], in_=ps2[:], func=Copy)
            pst = psum.tile([D, P], f32, tag="trans")
            nc.tensor.transpose(out=pst[:], in_=cp[:], identity=ident_sb[:])
            p2T = work.tile([D, P], f32, tag="p2T")
            nc.vector.tensor_copy(out=p2T[:], in_=pst[:])
            po = psum.tile([D, P], f32, tag="lout")
            nc.tensor.matmul(out=po[:], lhsT=w_sb["w2a"][:],
                             rhs=h1T_sb[:, t * P:(t + 1) * P],
                             start=True, stop=False)
            nc.tensor.matmul(out=po[:], lhsT=w_sb["w2b"][:],
                             rhs=tx1T_sb[:, t * P:(t + 1) * P],
                             start=False, stop=False)
            nc.tensor.matmul(out=po[:], lhsT=w_sb["w2c"][:], rhs=p2T[:],
                             start=False, stop=True)
            h2T = work.tile([D, P], f32, tag="h2T")
            nc.scalar.activation(out=h2T[:], in_=po[:], func=Relu,
                                 bias=b2_sb[:, 0:1])
            pl = psum1.tile([2, P], f32, tag="logit")
            nc.tensor.matmul(out=pl[:], lhsT=wlin_sb[:], rhs=h2T[:],
                             start=True, stop=True)
            nc.vector.tensor_scalar_add(
                out=logT_sb[:, t * P:(t + 1) * P], in0=pl[:],
                scalar1=blin_sb[:, 0:1])
        nc.sync.dma_start(out=y_out[:], in_=logT_sb[:])

        for _pool in (psum1, psum, oh_pool, work, const):
            _pool.release()

    nc.compile()
    return nc


# ----------------------------------------------------------------------------
# Entry point
# ----------------------------------------------------------------------------

def run_gnn(x, edge_index, W1, b1, W2, b2, Wlin, blin, trace=False):
    from concourse.bass_utils import run_bass_kernel_spmd

    global LAST_EXEC_NS
    x = np.asarray(x, np.float32)
    W1 = np.asarray(W1, np.float32)
    W2 = np.asarray(W2, np.float32)
    meta, dev = _preprocess(x, edge_index)
    nc = _build_program(meta)

    iota = np.broadcast_to(np.arange(P, dtype=np.float32), (P, P)).copy()
    ident = np.eye(P, dtype=np.float32)
    common = dict(
        xhat=dev["xhat"], iota=iota, ident=ident,
        w1a=np.ascontiguousarray(W1[0] - W1[2]), w1b=np.ascontiguousarray(W1[1]),
        w1c=np.ascontiguousarray(2.0 * W1[2]),
        w2a=np.ascontiguousarray(W2[0] - W2[2]), w2b=np.ascontiguousarray(W2[1]),
        w2c=np.ascontiguousarray(2.0 * W2[2]),
        wlin=np.asarray(Wlin, np.float32),
        b1=np.asarray(b1, np.float32).reshape(D, 1),
        b2=np.asarray(b2, np.float32).reshape(D, 1),
        blin=np.asarray(blin, np.float32).reshape(2, 1),
    )
    in_maps = [
        dict(common, idx=dev["idx"][c], colv=dev["colv"][c],
             negv=dev["negv"][c], dis_t=dev["dis_t"][c], xT=dev["xT"][c])
        for c in range(NCORE)
    ]
    global LAST_BUILD
    LAST_BUILD = (nc, in_maps)
    res = run_bass_kernel_spmd(nc, in_maps, list(range(NCORE)), trace=trace)
    LAST_EXEC_NS = res.exec_time_ns

    n, npc, T = meta["n"], meta["npc"], meta["T"]
    logits = np.zeros((n, 2), np.float32)
    r_all = np.arange(npc, dtype=np.int64)
    cols = (r_all % T) * P + r_all // T
    for c in range(NCORE):
        nodes = c * npc + r_all
        valid = nodes < n
        logits[nodes[valid]] = res.results[c]["y"][:, cols[valid]].T
    return logits


def kernel(**inputs):
    logits = run_gnn(
        inputs["x"], inputs["edge_index"], inputs["W1"], inputs["b1"],
        inputs["W2"], inputs["b2"], inputs["Wlin"], inputs["blin"],
        trace=TRACE,
    )
    return (logits, inputs["edge_index"])


# revision 15
# speedup vs baseline: 1.3418x; 1.3418x over previous
"""ChebConv(K=3) x2 + Linear GNN on 8 Trainium2 NeuronCores.

Strategy (graph/data parallel, per sharding hint):
  - Nodes are sharded by range across 8 cores (6250 real nodes/core, padded
    to 6272 = 49*128).  Each core owns the segment-sum outputs for its node
    range.  Chebyshev weights are replicated.
  - prop(z) = -D^-1/2 A D^-1/2 z is factored as  -diag(dis) * M(diag(dis) z)
    where M is the unweighted gather-sum over edges.  Source features are
    pre-scaled by dis before each AllGather, so the per-edge weight reduces
    to -dis[col], which is folded into the one-hot scatter matrix.
  - Per prop, each core dma_gathers its edges' source rows (256B each) from
    the replicated full feature table in HBM, then scatter-adds them into
    PSUM with one-hot matmuls on the TensorEngine (one-hots built on the
    VectorEngine from iota == col_local, scaled by -dis[col]).
  - Between props, the updated (pre-scaled) node features are AllGathered
    (3 collectives total).
  - dma_gather indices are int16, so the source table is split in two halves
    (< 32768 rows each); each tile's edges are grouped by (tile, half).
  - The schedule is uniform across cores (SPMD): per (tile, half) groups are
    padded to a common chunk cap; pad slots carry weight 0.
"""

import math
import os

import numpy as np

os.environ.setdefault("MYCRO_LOCAL_CACHE", "1")

P = 128          # partitions / chunk size
D = 64           # feature dim
NCORE = 8

# Set by test harness to get an NTFF profile; leave False for grading.
TRACE = False
LAST_EXEC_NS = None
LAST_BUILD = None   # (nc, in_maps) of the most recent run, for benchmarking


# ----------------------------------------------------------------------------
# Host-side preprocessing
# ----------------------------------------------------------------------------

def _preprocess(x, edge_index):
    """Build per-core edge streams + device input arrays."""
    n = x.shape[0]
    npc = (n + NCORE - 1) // NCORE          # real nodes per core
    t_per_core = (npc + P - 1) // P         # tiles per core
    T = t_per_core
    S = T * P                               # padded nodes per core
    NPTOT = S * NCORE
    HALF = (NCORE // 2) * S
    assert HALF < 32768 and NPTOT - HALF < 32768

    row = np.asarray(edge_index[0], dtype=np.int64)
    col = np.asarray(edge_index[1], dtype=np.int64)
    E = row.shape[0]

    deg = np.bincount(row, minlength=n).astype(np.float64)
    dis = np.where(deg > 0, 1.0 / np.sqrt(np.maximum(deg, 1.0)), 0.0).astype(
        np.float32
    )

    # node -> (core, tile t, partition p); padded slice row r = p*T + t
    core_of = col // npc
    lc = col - core_of * npc
    t_of = lc % T
    p_of = lc // T
    # padded global id of the source node (row-major: core*S + local)
    g_src = (row // npc) * S + (row % npc)
    h_of = (g_src >= HALF).astype(np.int64)
    w_e = (-dis[row] * dis[col]).astype(np.float32)

    group = (core_of * T + t_of) * 2 + h_of          # global group id
    order = np.argsort(group, kind="stable")
    cnt = np.bincount(group, minlength=NCORE * T * 2)
    cap_chunks = max(1, int(math.ceil(cnt.max() / P)))
    CAPS = cap_chunks * P                             # slots per group
    nch = 2 * T * cap_chunks                          # chunks per core
    slots = nch * P                                   # slots per core

    gidx = np.zeros((NCORE, slots), np.int16)
    colv = np.zeros((NCORE, slots), np.float32)
    negv = np.zeros((NCORE, slots), np.float32)

    g_sorted = group[order]
    starts = np.zeros(NCORE * T * 2 + 1, np.int64)
    np.cumsum(cnt, out=starts[1:])
    pos = np.arange(E, dtype=np.int64) - starts[g_sorted]
    core_s = g_sorted // (2 * T)
    slot = (g_sorted % (2 * T)) * CAPS + pos
    gidx[core_s, slot] = (g_src - h_of * HALF)[order].astype(np.int16)
    colv[core_s, slot] = p_of[order].astype(np.float32)
    negv[core_s, slot] = w_e[order]

    # device layouts
    idx_dev = np.zeros((NCORE, 128, nch * 8), np.int16)
    for c in range(NCORE):
        blocks = gidx[c].reshape(2 * T, CAPS // 16, 16)   # [group, s, 16]
        wrapped = blocks.transpose(0, 2, 1).reshape(2 * T, 16, CAPS // 16)
        flat = np.concatenate(list(wrapped), axis=1)      # [16, nch*8]
        idx_dev[c] = np.tile(flat, (8, 1))
    colv_dev = colv.reshape(NCORE, nch, P).transpose(0, 2, 1).copy()
    negv_dev = negv.reshape(NCORE, nch, P).transpose(0, 2, 1).copy()

    # per-core dis table [128, T] and xT [64, T*128] (tile-major columns)
    dis_t = np.zeros((NCORE, P, T), np.float32)
    xT = np.zeros((NCORE, D, S), np.float32)
    xhat = np.zeros((NPTOT, D), np.float32)
    r_all = np.arange(npc, dtype=np.int64)
    t_r = r_all % T
    p_r = r_all // T
    for c in range(NCORE):
        nodes = c * npc + r_all
        valid = nodes < n
        nv = nodes[valid]
        dis_t[c, p_r[valid], t_r[valid]] = dis[nv]
        xT[c][:, t_r[valid] * P + p_r[valid]] = np.asarray(x)[nv].T
        xhat[c * S + r_all[valid]] = np.asarray(x)[nv]

    meta = dict(
        n=n, npc=npc, T=T, S=S, NPTOT=NPTOT, HALF=HALF,
        cap=cap_chunks, nch=nch,
    )
    return meta, dict(
        idx=idx_dev, colv=colv_dev, negv=negv_dev, dis_t=dis_t, xT=xT,
        xhat=xhat,
    )


# ----------------------------------------------------------------------------
# Device program
# ----------------------------------------------------------------------------

def _build_program(meta):
    import concourse.bacc as bacc
    import concourse.mybir as mybir
    from concourse.tile import TileContext

    abl = os.environ.get("ABLATE", "")
    no_ag = "ag" in abl
    no_gather = "gather" in abl
    no_oh = "oh" in abl
    no_mm = "mm" in abl

    T, S, NPTOT, HALF = meta["T"], meta["S"], meta["NPTOT"], meta["HALF"]
    CAP, NCH = meta["cap"], meta["nch"]
    f32 = mybir.dt.float32

    nc = bacc.Bacc("TRN2", target_bir_lowering=False, debug=False,
                   num_devices=NCORE, num_swdge_queues=4)

    # I/O
    xhat_in = nc.dram_tensor("xhat", [NPTOT, D], f32, kind="ExternalInput")
    xT_in = nc.dram_tensor("xT", [D, S], f32, kind="ExternalInput")
    idx_in = nc.dram_tensor("idx", [128, NCH * 8], mybir.dt.int16,
                            kind="ExternalInput")
    colv_in = nc.dram_tensor("colv", [P, NCH], f32, kind="ExternalInput")
    negv_in = nc.dram_tensor("negv", [P, NCH], f32, kind="ExternalInput")
    dis_in = nc.dram_tensor("dis_t", [P, T], f32, kind="ExternalInput")
    iota_in = nc.dram_tensor("iota", [P, P], f32, kind="ExternalInput")
    ident_in = nc.dram_tensor("ident", [P, P], f32, kind="ExternalInput")
    w_in = {}
    for nm in ("w1a", "w1b", "w1c", "w2a", "w2b", "w2c"):
        w_in[nm] = nc.dram_tensor(nm, [D, D], f32, kind="ExternalInput")
    wlin_in = nc.dram_tensor("wlin", [D, 2], f32, kind="ExternalInput")
    b1_in = nc.dram_tensor("b1", [D, 1], f32, kind="ExternalInput")
    b2_in = nc.dram_tensor("b2", [D, 1], f32, kind="ExternalInput")
    blin_in = nc.dram_tensor("blin", [2, 1], f32, kind="ExternalInput")
    y_out = nc.dram_tensor("y", [2, S], f32, kind="ExternalOutput")

    # internal DRAM for collectives
    ag_in = [nc.dram_tensor(f"ag_in{i}", [S, D], f32) for i in range(3)]
    ag_out = [
        nc.dram_tensor(f"ag_out{i}", [NPTOT, D], f32, addr_space="Shared")
        for i in range(3)
    ]

    with TileContext(nc) as tc:
        const = tc.alloc_tile_pool(name="const", bufs=1)
        work = tc.alloc_tile_pool(name="work", bufs=2)
        oh_pool = tc.alloc_tile_pool(name="oh", bufs=4)
        psum = tc.alloc_tile_pool(name="psum", bufs=2, space="PSUM")
        psum1 = tc.alloc_tile_pool(name="psum1", bufs=1, space="PSUM")

        # persistent SBUF
        idx_sb = const.tile([128, NCH * 8], mybir.dt.int16)
        colv_sb = const.tile([P, NCH], f32)
        negv_sb = const.tile([P, NCH], f32)
        dis_sb = const.tile([P, T], f32)
        iota_sb = const.tile([P, P], f32)
        ident_sb = const.tile([P, P], f32)
        w_sb = {nm: const.tile([D, D], f32, name=f"w_{nm}") for nm in w_in}
        wlin_sb = const.tile([D, 2], f32)
        b1_sb = const.tile([D, 1], f32)
        b2_sb = const.tile([D, 1], f32)
        blin_sb = const.tile([2, 1], f32)
        tx1T_sb = const.tile([D, S], f32)
        h1T_sb = const.tile([D, S], f32)
        u_sb = const.tile([P, T * D], f32)
        logT_sb = const.tile([2, S], f32)

        nc.sync.dma_start(out=idx_sb[:], in_=idx_in[:])
        nc.sync.dma_start(out=colv_sb[:], in_=colv_in[:])
        nc.sync.dma_start(out=negv_sb[:], in_=negv_in[:])
        nc.sync.dma_start(out=dis_sb[:], in_=dis_in[:])
        nc.sync.dma_start(out=iota_sb[:], in_=iota_in[:])
        nc.sync.dma_start(out=ident_sb[:], in_=ident_in[:])
        for nm in w_in:
            nc.sync.dma_start(out=w_sb[nm][:], in_=w_in[nm][:])
        nc.sync.dma_start(out=wlin_sb[:], in_=wlin_in[:])
        nc.sync.dma_start(out=b1_sb[:], in_=b1_in[:])
        nc.sync.dma_start(out=b2_sb[:], in_=b2_in[:])
        nc.sync.dma_start(out=blin_sb[:], in_=blin_in[:])

        Copy = mybir.ActivationFunctionType.Copy
        Relu = mybir.ActivationFunctionType.Relu

        def scatter_prop_tile(src, t):
            """Gather + one-hot scatter matmul for tile t -> psum [128, 64].

            src: DRAM AP [NPTOT, D] to gather from (pre-scaled by dis).
            Returns the psum tile (= -dis[col] * sum(src[row])).
            """
            gbuf = work.tile([P, 2 * CAP, D], f32, tag="gbuf")
            lo = src[0:HALF, :]
            hi = src[HALF:NPTOT, :]
            base = (2 * t) * CAP * 8
            if not no_gather:
                nc.gpsimd.dma_gather(
                    out_ap=gbuf[:, 0:CAP, :], in_ap=lo,
                    idxs_ap=idx_sb[:, base:base + CAP * 8],
                    num_idxs=CAP * P, num_idxs_reg=CAP * P, elem_size=D,
                    single_packet=False, queue_num=(2 * t) % 4,
                )
                nc.gpsimd.dma_gather(
                    out_ap=gbuf[:, CAP:2 * CAP, :], in_ap=hi,
                    idxs_ap=idx_sb[:, base + CAP * 8:base + 2 * CAP * 8],
                    num_idxs=CAP * P, num_idxs_reg=CAP * P, elem_size=D,
                    single_packet=False, queue_num=(2 * t + 1) % 4,
                )
            ps = psum.tile([P, D], f32, tag="scatter")
            ch0 = 2 * t * CAP
            for k in range(2 * CAP):
                if not no_oh:
                    oh = oh_pool.tile([P, P], f32, tag="oh")
                    nc.vector.tensor_scalar(
                        out=oh[:], in0=iota_sb[:],
                        scalar1=colv_sb[:, ch0 + k:ch0 + k + 1],
                        scalar2=negv_sb[:, ch0 + k:ch0 + k + 1],
                        op0=mybir.AluOpType.is_equal,
                        op1=mybir.AluOpType.mult,
                    )
                else:
                    oh = iota_sb
                if not no_mm:
                    nc.tensor.matmul(
                        out=ps[:], lhsT=oh[:], rhs=gbuf[:, k, :],
                        start=(k == 0), stop=(k == 2 * CAP - 1),
                    )
            if no_mm:
                nc.tensor.matmul(out=ps[:], lhsT=iota_sb[:, :P],
                                 rhs=gbuf[:, 0, :], start=True, stop=True)
            return ps

        def transpose_to(ps_nodemajor, t, dest_sb):
            """psum [128,64] (node-major) -> dest_sb[:, t*128:(t+1)*128]."""
            cp = work.tile([P, D], f32, tag="cp")
            nc.scalar.activation(out=cp[:], in_=ps_nodemajor[:], func=Copy)
            pst = psum.tile([D, P], f32, tag="trans")
            nc.tensor.transpose(out=pst[:], in_=cp[:], identity=ident_sb[:])
            nc.scalar.activation(
                out=dest_sb[:, t * P:(t + 1) * P], in_=pst[:], func=Copy)

        def layer_out_tile(t, wa, wb, wc, p2T_sb, xT_t, bias_sb, dest_sb):
            """dest_sb[:, tile t] = relu((wa.T@xT + wb.T@tx1T + wc.T@p2T) + b)."""
            po = psum.tile([D, P], f32, tag="lout")
            nc.tensor.matmul(out=po[:], lhsT=w_sb[wa][:], rhs=xT_t,
                             start=True, stop=False)
            nc.tensor.matmul(out=po[:], lhsT=w_sb[wb][:],
                             rhs=tx1T_sb[:, t * P:(t + 1) * P],
                             start=False, stop=False)
            nc.tensor.matmul(out=po[:], lhsT=w_sb[wc][:], rhs=p2T_sb[:],
                             start=False, stop=True)
            nc.scalar.activation(
                out=dest_sb[:, t * P:(t + 1) * P],
                in_=po[:], func=Relu, bias=bias_sb[:, 0:1])

        def prop_pass(src, to_txT, stage_u):
            """One prop over all tiles.

            to_txT: SBUF [64, S] to store transposed prop result, or None.
            stage_u: if True, stage dis*psum into u_sb (for AllGather).
            """
            for t in range(T):
                ps = scatter_prop_tile(src, t)
                if to_txT is not None:
                    transpose_to(ps, t, to_txT)
                if stage_u:
                    nc.scalar.activation(
                        out=u_sb[:, t * D:(t + 1) * D], in_=ps[:], func=Copy)

        def allgather(i):
            nc.sync.dma_start(
                out=ag_in[i][:].rearrange("(p t) f -> p (t f)", p=P),
                in_=u_sb[:])
            if no_ag:
                nc.sync.dma_start(
                    out=ag_out[i][0:S, :].rearrange("(p t) f -> p (t f)", p=P),
                    in_=u_sb[:])
            else:
                nc.gpsimd.collective_compute(
                    "AllGather", mybir.AluOpType.bypass,
                    replica_groups=[list(range(NCORE))],
                    ins=[ag_in[i][:]], outs=[ag_out[i][:]],
                )

        # ---- layer 1 ----
        prop_pass(xhat_in[:], tx1T_sb, stage_u=True)
        allgather(0)
        # prop2 + layer-1 output, fused per tile
        for t in range(T):
            ps2 = scatter_prop_tile(ag_out[0][:], t)
            cp = work.tile([P, D], f32, tag="cp")
            nc.scalar.activation(out=cp[:], in_=ps2[:], func=Copy)
            pst = psum.tile([D, P], f32, tag="trans")
            nc.tensor.transpose(out=pst[:], in_=cp[:], identity=ident_sb[:])
            p2T = work.tile([D, P], f32, tag="p2T")
            nc.vector.tensor_copy(out=p2T[:], in_=pst[:])
            xT_t = work.tile([D, P], f32, tag="xTt")
            nc.sync.dma_start(out=xT_t[:], in_=xT_in[:, t * P:(t + 1) * P])
            layer_out_tile(t, "w1a", "w1b", "w1c", p2T, xT_t[:], b1_sb,
                           h1T_sb)
            # u1 = dis * h1 (node-major) for next AllGather
            psn = psum1.tile([P, D], f32, tag="nodemaj")
            nc.tensor.matmul(
                out=psn[:], lhsT=h1T_sb[:, t * P:(t + 1) * P],
                rhs=ident_sb[:D, :D], is_transpose=True)
            nc.scalar.activation(
                out=u_sb[:, t * D:(t + 1) * D], in_=psn[:], func=Copy)
        allgather(1)

        # ---- layer 2 ----
        prop_pass(ag_out[1][:], tx1T_sb, stage_u=True)
        allgather(2)
        for t in range(T):
            ps2 = scatter_prop_tile(ag_out[2][:], t)
            cp = work.tile([P, D], f32, tag="cp")
            nc.scalar.activation(out=cp[:], in_=ps2[:], func=Copy)
            pst = psum.tile([D, P], f32, tag="trans")
            nc.tensor.transpose(out=pst[:], in_=cp[:], identity=ident_sb[:])
            p2T = work.tile([D, P], f32, tag="p2T")
            nc.vector.tensor_copy(out=p2T[:], in_=pst[:])
            po = psum.tile([D, P], f32, tag="lout")
            nc.tensor.matmul(out=po[:], lhsT=w_sb["w2a"][:],
                             rhs=h1T_sb[:, t * P:(t + 1) * P],
                             start=True, stop=False)
            nc.tensor.matmul(out=po[:], lhsT=w_sb["w2b"][:],
                             rhs=tx1T_sb[:, t * P:(t + 1) * P],
                             start=False, stop=False)
            nc.tensor.matmul(out=po[:], lhsT=w_sb["w2c"][:], rhs=p2T[:],
                             start=False, stop=True)
            h2T = work.tile([D, P], f32, tag="h2T")
            nc.scalar.activation(out=h2T[:], in_=po[:], func=Relu,
                                 bias=b2_sb[:, 0:1])
            pl = psum1.tile([2, P], f32, tag="logit")
            nc.tensor.matmul(out=pl[:], lhsT=wlin_sb[:], rhs=h2T[:],
                             start=True, stop=True)
            nc.vector.tensor_scalar_add(
                out=logT_sb[:, t * P:(t + 1) * P], in0=pl[:],
                scalar1=blin_sb[:, 0:1])
        nc.sync.dma_start(out=y_out[:], in_=logT_sb[:])

        for _pool in (psum1, psum, oh_pool, work, const):
            _pool.release()

    nc.compile()
    return nc


# ----------------------------------------------------------------------------
# Entry point
# ----------------------------------------------------------------------------

def run_gnn(x, edge_index, W1, b1, W2, b2, Wlin, blin, trace=False):
    from concourse.bass_utils import run_bass_kernel_spmd

    global LAST_EXEC_NS
    x = np.asarray(x, np.float32)
    W1 = np.asarray(W1, np.float32)
    W2 = np.asarray(W2, np.float32)
    meta, dev = _preprocess(x, edge_index)
    nc = _build_program(meta)

    iota = np.broadcast_to(np.arange(P, dtype=np.float32), (P, P)).copy()
    ident = np.eye(P, dtype=np.float32)
    common = dict(
        xhat=dev["xhat"], iota=iota, ident=ident,
        w1a=np.ascontiguousarray(W1[0] - W1[2]), w1b=np.ascontiguousarray(W1[1]),
        w1c=np.ascontiguousarray(2.0 * W1[2]),
        w2a=np.ascontiguousarray(W2[0] - W2[2]), w2b=np.ascontiguousarray(W2[1]),
        w2c=np.ascontiguousarray(2.0 * W2[2]),
        wlin=np.asarray(Wlin, np.float32),
        b1=np.asarray(b1, np.float32).reshape(D, 1),
        b2=np.asarray(b2, np.float32).reshape(D, 1),
        blin=np.asarray(blin, np.float32).reshape(2, 1),
    )
    in_maps = [
        dict(common, idx=dev["idx"][c], colv=dev["colv"][c],
             negv=dev["negv"][c], dis_t=dev["dis_t"][c], xT=dev["xT"][c])
        for c in range(NCORE)
    ]
    global LAST_BUILD
    LAST_BUILD = (nc, in_maps)
    res = run_bass_kernel_spmd(nc, in_maps, list(range(NCORE)), trace=trace)
    LAST_EXEC_NS = res.exec_time_ns

    n, npc, T = meta["n"], meta["npc"], meta["T"]
    logits = np.zeros((n, 2), np.float32)
    r_all = np.arange(npc, dtype=np.int64)
    cols = (r_all % T) * P + r_all // T
    for c in range(NCORE):
        nodes = c * npc + r_all
        valid = nodes < n
        logits[nodes[valid]] = res.results[c]["y"][:, cols[valid]].T
    return logits


def kernel(**inputs):
    logits = run_gnn(
        inputs["x"], inputs["edge_index"], inputs["W1"], inputs["b1"],
        inputs["W2"], inputs["b2"], inputs["Wlin"], inputs["blin"],
        trace=TRACE,
    )
    return (logits, inputs["edge_index"])
